# revision 21
# baseline (speedup 1.0000x reference)
"""Trainium2 Bass kernel for nn_AttentionLayer (sparse_attention).

Reference computation (per batch b):
    q = wq @ x + bq          [8, N]     (1x1 conv, d=8, N=H*W=4096)
    k = wk @ x + bk          [8, N]
    v = wv @ x + bv          [64, N]
    energy = q^T k           [N, N]
    attn = softmax(energy, axis=-1)
    out = gamma * (v @ attn^T) + x

Sharding: data-parallel over batch; 8 batches -> 8 NeuronCores, one batch
element per core.  Weights replicated.  No collectives.

Architecture (hardware-measured rates drove every choice):
  - PE moving-operand SBUF read BW (~494 GB/s) caps matmul issue: a
    [*,512]-col matmul takes 259 ns in bf16 (128 KB moving) but 216 ns in
    plain fp8 (64 KB, clock-bound).  DoubleRow fp8 doubles CONTRACTION per
    column (2 k-slabs), not column rate.
  - Energy: plain fp8e4 matmuls.  q,k scaled by 1/4, replicated 16x over
    the 128 contraction partitions (weights pre-replicated), so psum =
    16*(q/4 . k/4) = q.k exactly.  64 MMs/pair @ ~216 ns = 13.8 us.
  - Out (v @ attn^T): DoubleRow fp8: lhsT = vT jb-pair [128, 2, 128]
    (slab stride MUST be 128 - ISA dual-fp8 LDW restriction), rhs = aT
    [128, 2, 512].  One MM covers TWO j-blocks: 32 MMs/pair ~ 9 us.
    vT col 64 = ones => psum row 64 accumulates the softmax denominator.
  - exp: energies shifted by -3 (softmax-invariant) so exp in [e-12, 4.5].
    Split ACT (true exp -> fp8e5, bias=-3, ~1.37 us/[128,1024] tile) and
    DVE (Schraudolph in e5m2 bit domain: i8(round(A*e + B)), one
    tensor_scalar, ~1.2 us) - both read f32 PSUM, the hard wall (GPSIMD
    cannot access PSUM).
  - normalize on Pool (gpsimd), which cannot touch psum but is idle
    otherwise: yu = ACT copy psum->bf16; 1/s via bf16 magic seed + one
    Newton step; r broadcast over partitions via a DRAM round-trip DMA;
    y = x - yu*(-r) with the residual add in f32 (bf16 add costs 6e-3
    rel err; f32 keeps it at ~2e-3).  Last pair's chain runs on DVE.

Accuracy: attention term is ~1% of output; fp8 q/k/v + e5m2 attn weights
+ bf16 normalize cost ~2e-3 final relative error (tolerance 2e-2).
"""

import os
import sys

import numpy as np

sys.path.insert(0, "/opt/trn_rl_repo")

B, C, HH, WW = 8, 64, 64, 64
N = HH * WW  # 4096
D = 8  # qk channels
IC = 512  # i-chunk
N_IC = N // IC  # 8
JB = 128  # j-block
N_JB = N // JB  # 32
NP = N_JB // 2  # 16 jb-pairs

A5 = float(4.0 / np.log(2.0))
B5 = float(4 * 15.0 - 0.5)
SQK = float(0.25 * np.sqrt(A5))  # per-side scale: 16 reps * SQK^2 = A5
K16 = 0x7EF0  # bf16 reciprocal magic
PIPE = 3

# jb's whose exp runs on ACT (17, evenly spread); rest on DVE (15)
ACT_SET = frozenset(j for j in range(32) if (j * 17) // 32 != ((j + 1) * 17) // 32)

_CACHE = {}


def _build_program():
    import concourse.bass as bass
    import concourse.tile as tile
    from concourse import bacc, mybir
    from concourse.masks import make_identity

    f32 = mybir.dt.float32
    bf16 = mybir.dt.bfloat16
    i8 = mybir.dt.int8
    i16 = mybir.dt.int16
    fp8e4 = mybir.dt.float8e4
    fp8e5 = mybir.dt.float8e5
    EXP = mybir.ActivationFunctionType.Exp
    DR = mybir.MatmulPerfMode.DoubleRow
    MUL = mybir.AluOpType.mult
    ADD = mybir.AluOpType.add
    SUB = mybir.AluOpType.subtract

    nc = bacc.Bacc(
        "TRN2", target_bir_lowering=False, debug=False, enable_asserts=False
    )

    x_d = nc.dram_tensor("x", [C, N], f32, kind="ExternalInput").ap()
    wq_d = nc.dram_tensor("wq", [D, C], f32, kind="ExternalInput").ap()
    bq_d = nc.dram_tensor("bq", [D], f32, kind="ExternalInput").ap()
    wk_d = nc.dram_tensor("wk", [D, C], f32, kind="ExternalInput").ap()
    bk_d = nc.dram_tensor("bk", [D], f32, kind="ExternalInput").ap()
    wv_d = nc.dram_tensor("wv", [C, C], f32, kind="ExternalInput").ap()
    bv_d = nc.dram_tensor("bv", [C], f32, kind="ExternalInput").ap()
    gamma_d = nc.dram_tensor("gamma", [1], f32, kind="ExternalInput").ap()
    y_d = nc.dram_tensor("y", [C, N], f32, kind="ExternalOutput").ap()
    r_d = nc.dram_tensor("r_scr", [5, 2 * IC], bf16, kind="Internal").ap()

    with tile.TileContext(nc) as tc:
        from contextlib import ExitStack

        with ExitStack() as ctx:
            consts = ctx.enter_context(tc.tile_pool(name="consts", bufs=1))
            bigs = ctx.enter_context(tc.tile_pool(name="bigs", bufs=1))
            work = ctx.enter_context(tc.tile_pool(name="work", bufs=4))
            ypool = ctx.enter_context(tc.tile_pool(name="ypool", bufs=2))
            small = ctx.enter_context(tc.tile_pool(name="small", bufs=4))

            # ---------------- constants / weights prep ----------------
            ident = consts.tile([C, C], f32)
            make_identity(nc, ident)

            # warm the Exp activation table early (table load ~1.3us)
            warm = consts.tile([1, 8], f32)
            nc.scalar.activation(warm, ident[0:1, 0:8], EXP)

            mones = consts.tile([65, C], bf16)
            nc.vector.memset(mones, -1.0)

            gcol = consts.tile([65, 1], f32)
            nc.sync.dma_start(out=gcol, in_=gamma_d.to_broadcast([65, 1]))

            # x2c: 8 per-chunk tiles [128, 512] bf16 (separate tiles so
            # each projection matmul waits only its own chunk's DMA, not a
            # pooled queue semaphore).  rows 0:64 = x (casting DMA), row 64
            # = ones (tiny DMA from scratch), rows 65:127 = zero (per-chunk
            # DVE memset) so K=128 keeps the PE at full clock.
            pones = consts.tile([1, IC], bf16)
            nc.vector.memset(pones, 1.0)
            nc.sync.dma_start(out=r_d[4:5, 0:IC], in_=pones)
            x2c = []
            for ic in range(N_IC):
                t = bigs.tile([2 * C, IC], bf16, name=f"x2c{ic}")
                x2c.append(t)
                sl = slice(ic * IC, (ic + 1) * IC)
                nc.gpsimd.dma_start(out=t[0:C, :], in_=x_d[:, sl])
                nc.vector.memset(t[C : 2 * C, :], 0.0)
                nc.sync.dma_start(out=t[C : C + 1, :], in_=r_d[4:5, 0:IC])
            # x_f32 for the final residual add
            xf32 = bigs.tile([C, N], f32)

            wq_sb = consts.tile([D, C], f32)
            wk_sb = consts.tile([D, C], f32)
            wv_sb = consts.tile([C, C], f32)
            nc.sync.dma_start(out=wq_sb, in_=wq_d)
            nc.sync.dma_start(out=wk_sb, in_=wk_d)
            nc.sync.dma_start(out=wv_sb, in_=wv_d)

            # fold gamma into wv; SQK into wq and wk (16 replicas of
            # (SQK q).(SQK k) summed over K give A5 * q.k, so the
            # Schraudolph multiply is free and ACT applies scale=1/A5)
            nc.vector.tensor_scalar_mul(wv_sb, wv_sb, gcol[0:C])
            nc.vector.tensor_scalar_mul(wq_sb, wq_sb, SQK)
            nc.vector.tensor_scalar_mul(wk_sb, wk_sb, SQK)

            wqT = consts.tile([2 * C, 2 * C], bf16)
            wkT = consts.tile([2 * C, 2 * C], bf16)
            wvT2 = consts.tile([2 * C, C + 1], bf16)
            nc.vector.memset(wqT, 0.0)
            nc.vector.memset(wkT, 0.0)
            nc.vector.memset(wvT2, 0.0)

            # vT: [128, NP, 2, 128] fp8e4; [j, p, s, 0:64] = gamma*v^T for
            # j-block 2p+s, col 64 = ones (denominator), cols 65:127 = 0
            # (psum rows 65:127 are never read, but keep them finite).
            vT = bigs.tile([JB, NP, 2, JB], fp8e4)
            nc.gpsimd.memset(vT, 0.0)
            nc.gpsimd.memset(vT[:, :, :, C : C + 1], 1.0)

            with tc.tile_pool(name="psum_x", bufs=4, space="PSUM") as psum_x:
                # biases staged on partition 64
                bst64 = consts.tile([65, 2 * D + C], f32)
                nc.sync.dma_start(out=bst64[C : C + 1, 0:D], in_=bq_d[None, :])
                nc.sync.dma_start(
                    out=bst64[C : C + 1, D : 2 * D], in_=bk_d[None, :]
                )
                nc.sync.dma_start(
                    out=bst64[C : C + 1, 2 * D :], in_=bv_d[None, :]
                )
                nc.vector.tensor_scalar_mul(
                    bst64[C : C + 1, 0 : 2 * D], bst64[C : C + 1, 0 : 2 * D],
                    SQK,
                )
                nc.vector.tensor_scalar_mul(
                    bst64[C : C + 1, 2 * D :], bst64[C : C + 1, 2 * D :],
                    gcol[C : C + 1],
                )

                nc.sync.dma_start(out=xf32[:, 0 : N // 2], in_=x_d[:, 0 : N // 2])
                nc.sync.dma_start(out=xf32[:, N // 2 :], in_=x_d[:, N // 2 :])

                wqT8 = consts.tile([65, D], bf16)
                pt = psum_x.tile([C, D], f32, tag="px")
                nc.tensor.transpose(pt, wq_sb, ident[0:D, 0:D])
                nc.vector.tensor_copy(out=wqT8[0:C, :], in_=pt)
                nc.vector.tensor_copy(
                    out=wqT8[C : C + 1, :], in_=bst64[C : C + 1, 0:D]
                )

                wkT8 = consts.tile([65, D], bf16)
                pt2 = psum_x.tile([C, D], f32, tag="px")
                nc.tensor.transpose(pt2, wk_sb, ident[0:D, 0:D])
                nc.vector.tensor_copy(out=wkT8[0:C, :], in_=pt2)
                nc.vector.tensor_copy(
                    out=wkT8[C : C + 1, :], in_=bst64[C : C + 1, D : 2 * D]
                )

                # replicate 16x across the 128 weight columns
                for w_dst, w_src in ((wqT, wqT8), (wkT, wkT8)):
                    sap = w_src[:]
                    rep = bass.AP(
                        tensor=sap.tensor,
                        offset=sap.offset,
                        ap=[sap.ap[0], [0, 16], sap.ap[1]],
                    )
                    nc.vector.tensor_copy(
                        out=w_dst[0:65].rearrange("p (g d) -> p g d", g=16),
                        in_=rep,
                    )

                # wvT2 [128, 65]: rows 0:64 = (gamma wv)^T, row 64 = gamma bv
                pt3 = psum_x.tile([C, C], f32, tag="px")
                nc.tensor.transpose(pt3, wv_sb, ident)
                nc.vector.tensor_copy(out=wvT2[0:C, 0:C], in_=pt3)
                nc.vector.tensor_copy(
                    out=wvT2[C : C + 1, 0:C], in_=bst64[C : C + 1, 2 * D :]
                )

                # ---------------- projections ----------------
                # q8/k8: [128, N] fp8e4 (16 partition-replicas of q/4, k/4)
                # double-width psum tiles amortize copy overhead; copies
                # alternate ACT/DVE so neither serializes the prep.
                q8 = bigs.tile([2 * C, N], fp8e4)
                k8 = bigs.tile([2 * C, N], fp8e4)
                for ic in range(N_IC):
                    sl = slice(ic * IC, (ic + 1) * IC)
                    pq = psum_x.tile([2 * C, IC], f32, tag="px")
                    nc.tensor.matmul(pq, wqT, x2c[ic][:], start=True, stop=True)
                    nc.vector.tensor_copy(out=q8[:, sl], in_=pq)
                    pk = psum_x.tile([2 * C, IC], f32, tag="px")
                    nc.tensor.matmul(pk, wkT, x2c[ic][:], start=True, stop=True)
                    nc.vector.tensor_copy(out=k8[:, sl], in_=pk)
                    pv = psum_x.tile([JB, 4 * 65], f32, tag="pv", bufs=4)
                    for j4 in range(4):
                        nc.tensor.matmul(
                            pv[:, j4 * 65 : (j4 + 1) * 65],
                            x2c[ic][:, j4 * JB : (j4 + 1) * JB],
                            wvT2,
                            start=True,
                            stop=True,
                        )
                    nc.scalar.copy(
                        vT[:, 2 * ic : 2 * ic + 2, :, 0:C],
                        pv.rearrange("p (a b f) -> p a b f", a=2, b=2)[
                            :, :, :, 0:C
                        ],
                    )

            psum_e = ctx.enter_context(
                tc.tile_pool(name="psum_e", bufs=3, space="PSUM")
            )
            psum_o = ctx.enter_context(
                tc.tile_pool(name="psum_o", bufs=1, space="PSUM")
            )

            # ---------------- main attention loop ----------------
            # Deferred normalize: pair pr's chain is emitted early in pair
            # pr+1 (Pool + DMA only; nothing the PE waits on).  The last
            # pair's chain runs on DVE after the loop.
            norm_q = []

            def emit_norm(yu, sl2, pr):
                # mid-loop normalize (pairs 0..2): seed on DVE (i16 TS is
                # DVE-only), Newton + big TTs on Pool, partition-broadcast
                # of r via a DRAM round-trip.
                # r0 = +1/s seed: bitcast_bf16(K16 - int16(s_bits))
                r0i = small.tile([C + 1, 2 * IC], i16, tag="r0")
                nc.vector.tensor_scalar(
                    r0i[C : C + 1, :],
                    yu[C : C + 1, :].bitcast(i16),
                    -1.0,
                    float(K16),
                    op0=MUL,
                    op1=ADD,
                )
                r0 = r0i.bitcast(bf16)
                # one Newton step, lands NEGATED: rn = (s*r0 - 2)*r0 = -1/s
                t1 = small.tile([C + 1, 2 * IC], bf16, tag="t1")
                nc.gpsimd.tensor_tensor(
                    out=t1[C : C + 1, :], in0=yu[C : C + 1, :],
                    in1=r0[C : C + 1, :], op=MUL,
                )
                u = small.tile([C + 1, 2 * IC], bf16, tag="u")
                nc.gpsimd.tensor_scalar(
                    u[C : C + 1, :], t1[C : C + 1, :], 1.0, -2.0,
                    op0=MUL, op1=ADD,
                )
                rn = small.tile([C + 1, 2 * IC], bf16, tag="rn")
                nc.gpsimd.tensor_tensor(
                    out=rn[C : C + 1, :], in0=u[C : C + 1, :],
                    in1=r0[C : C + 1, :], op=MUL,
                )
                # broadcast -r over 64 partitions via DRAM round-trip
                nc.sync.dma_start(out=r_d[pr : pr + 1, :], in_=rn[C : C + 1, :])
                rb = small.tile([C, 2 * IC], bf16, tag="rb")
                nc.sync.dma_start(
                    out=rb, in_=r_d[pr : pr + 1, :].to_broadcast([C, 2 * IC])
                )
                # t = yu * (-r);  y = x - t  (f32 residual add)
                t2 = small.tile([C, 2 * IC], bf16, tag="t2")
                nc.gpsimd.tensor_tensor(out=t2, in0=yu[0:C, :], in1=rb, op=MUL)
                y_sb = ypool.tile([C, 2 * IC], f32)
                nc.gpsimd.tensor_tensor(
                    out=y_sb, in0=xf32[:, sl2], in1=t2, op=SUB
                )
                nc.sync.dma_start(out=y_d[:, sl2], in_=y_sb)

            def emit_norm_tail(yu, sl2):
                # last pair: latency-optimal.  Two independent half-chains
                # (the idle-PE downclock halves tail op rates, so overlap
                # ACT/DVE/PE work).  Magic seed only, K=1 matmul broadcast.
                for h in range(2):
                    hs = slice(h * IC, (h + 1) * IC)
                    hs2 = slice(sl2.start + h * IC, sl2.start + (h + 1) * IC)
                    r0i = small.tile([C + 1, IC], i16, tag=f"r0t{h}")
                    nc.vector.tensor_scalar(
                        r0i[C : C + 1, :],
                        yu[C : C + 1, hs].bitcast(i16),
                        -1.0,
                        float(K16),
                        op0=MUL,
                        op1=ADD,
                    )
                    r0 = r0i.bitcast(bf16)
                    rb_ps = psum_e.tile([JB, 2 * IC], f32, tag="e_ps")
                    nc.tensor.matmul(
                        rb_ps[0:C, 0:IC], mones[C : C + 1, :],
                        r0[C : C + 1, :], start=True, stop=True,
                    )
                    t2 = small.tile([C, IC], bf16, tag=f"t2t{h}")
                    nc.vector.tensor_tensor(
                        out=t2, in0=yu[0:C, hs], in1=rb_ps[0:C, 0:IC], op=MUL
                    )
                    y_sb = ypool.tile([C, IC], f32)
                    nc.vector.tensor_tensor(
                        out=y_sb, in0=xf32[:, hs2], in1=t2, op=SUB
                    )
                    nc.sync.dma_start(out=y_d[:, hs2], in_=y_sb)

            # Flat jb-stream across all 4 i-chunk pairs: the next pair's
            # energy matmuls fill the PE wait on the previous pair's tail
            # exps (no per-pair boundary stall).
            NPAIRS = N_IC // 2
            NT = NPAIRS * N_JB
            o_tiles = {}
            a_tiles = {}
            for g in range(NT + PIPE + 1):
                pr, jb = divmod(g, N_JB)
                if norm_q and jb == 1:
                    norm_q.pop(0)()
                if g < NT:
                    slA = slice((2 * pr) * IC, (2 * pr + 1) * IC)
                    slB = slice((2 * pr + 1) * IC, (2 * pr + 2) * IC)
                    e_ps = psum_e.tile([JB, 2 * IC], f32)
                    kblk = k8[:, jb * JB : (jb + 1) * JB]
                    nc.tensor.matmul(
                        e_ps[:, 0:IC], kblk, q8[:, slA],
                        start=True, stop=True,
                    )
                    nc.tensor.matmul(
                        e_ps[:, IC : 2 * IC], kblk, q8[:, slB],
                        start=True, stop=True,
                    )
                    p = jb // 2
                    if jb % 2 == 0:
                        aT_new = work.tile([JB, 2, 2 * IC], fp8e5, tag="aT")
                        a_tiles[(pr, p)] = aT_new
                    if jb in ACT_SET:
                        nc.scalar.activation(
                            a_tiles[(pr, p)][:, jb % 2, :], e_ps, EXP,
                            scale=float(1.0 / A5),
                        )
                    else:
                        nc.vector.tensor_scalar(
                            a_tiles[(pr, p)][:, jb % 2, :].bitcast(i8),
                            e_ps,
                            B5,
                            None,
                            op0=ADD,
                        )
                go = g - PIPE
                if 0 <= go < NT:
                    pro, jo = divmod(go, N_JB)
                    if jo % 2 == 1:
                        p = jo // 2
                        if p == 0:
                            o_new = psum_o.tile(
                                [2 * C, 2 * IC], f32, tag="op"
                            )
                            o_tiles[pro] = o_new
                        o_ps = o_tiles[p if False else pro]
                        aT = a_tiles.pop((pro, p))
                        nc.tensor.matmul(
                            o_ps[:, 0:IC],
                            vT[:, p],
                            aT[:, :, 0:IC],
                            start=(p == 0),
                            stop=(p == NP - 1),
                            perf_mode=DR,
                        )
                        nc.tensor.matmul(
                            o_ps[:, IC : 2 * IC],
                            vT[:, p],
                            aT[:, :, IC : 2 * IC],
                            start=(p == 0),
                            stop=(p == NP - 1),
                            perf_mode=DR,
                        )
                        if p == NP - 1:
                            # evacuate rows 0:65 to bf16 on ACT; frees the
                            # psum banks for the next pair's accumulator
                            o_done = o_tiles.pop(pro)
                            yu = small.tile([C + 1, 2 * IC], bf16, tag="yu")
                            nc.scalar.copy(yu, o_done[0 : C + 1, :])
                            sl2 = slice(
                                (2 * pro) * IC, (2 * pro + 2) * IC
                            )
                            if pro < NPAIRS - 1:
                                norm_q.append(
                                    lambda yu=yu, sl2=sl2, pro=pro: emit_norm(
                                        yu, sl2, pro
                                    )
                                )
                            else:
                                emit_norm_tail(yu, sl2)

            while norm_q:
                norm_q.pop(0)()

    nc.compile()
    return nc


def _get_program():
    if "nc" not in _CACHE:
        _CACHE["nc"] = _build_program()
    return _CACHE["nc"]


def kernel(**inputs) -> np.ndarray:
    import time

    nc = _get_program()
    from concourse.bass_utils import run_bass_kernel_spmd

    x = np.ascontiguousarray(np.asarray(inputs["x"], dtype=np.float32))
    shared = {
        k: np.ascontiguousarray(np.asarray(inputs[k], dtype=np.float32))
        for k in ("wq", "bq", "wk", "bk", "wv", "bv", "gamma")
    }
    in_maps = [
        {"x": x[b].reshape(C, N).copy(), **shared} for b in range(B)
    ]
    # the axon-tunneled device occasionally reports a transient
    # NRT_EXEC_UNIT_UNRECOVERABLE; a retry on a fresh execution succeeds
    last_err = None
    for attempt in range(4):
        try:
            res = run_bass_kernel_spmd(nc, in_maps, list(range(B)))
            break
        except Exception as e:  # noqa: BLE001
            last_err = e
            time.sleep(2.0 * (attempt + 1))
    else:
        raise last_err
    out = np.stack(
        [res.results[b]["y"].reshape(C, HH, WW) for b in range(B)], axis=0
    )
    return out.astype(np.float32)


if __name__ == "__main__":
    rng = np.random.default_rng(0)
    inputs = {
        "x": rng.standard_normal((B, C, HH, WW), dtype=np.float32),
        "wq": rng.standard_normal((D, C), dtype=np.float32) * 0.05,
        "bq": rng.standard_normal((D,), dtype=np.float32) * 0.05,
        "wk": rng.standard_normal((D, C), dtype=np.float32) * 0.05,
        "bk": rng.standard_normal((D,), dtype=np.float32) * 0.05,
        "wv": rng.standard_normal((C, C), dtype=np.float32) * 0.05,
        "bv": rng.standard_normal((C,), dtype=np.float32) * 0.05,
        "gamma": rng.standard_normal((1,), dtype=np.float32),
    }
    out = kernel(**inputs)
    print("out", out.shape, out.dtype, float(np.abs(out).max()))


# revision 23
# speedup vs baseline: 1.0002x; 1.0002x over previous
"""Trainium2 Bass kernel for nn_AttentionLayer (sparse_attention).

Reference computation (per batch b):
    q = wq @ x + bq          [8, N]     (1x1 conv, d=8, N=H*W=4096)
    k = wk @ x + bk          [8, N]
    v = wv @ x + bv          [64, N]
    energy = q^T k           [N, N]
    attn = softmax(energy, axis=-1)
    out = gamma * (v @ attn^T) + x

Sharding: data-parallel over batch; 8 batches -> 8 NeuronCores, one batch
element per core.  Weights replicated.  No collectives.

Architecture (hardware-measured rates drove every choice):
  - PE moving-operand SBUF read BW (~494 GB/s) caps matmul issue: a
    [*,512]-col matmul takes 259 ns in bf16 (128 KB moving) but 216 ns in
    plain fp8 (64 KB, clock-bound).  DoubleRow fp8 doubles CONTRACTION per
    column (2 k-slabs), not column rate.
  - Energy: plain fp8e4 matmuls.  q,k scaled by 1/4, replicated 16x over
    the 128 contraction partitions (weights pre-replicated), so psum =
    16*(q/4 . k/4) = q.k exactly.  64 MMs/pair @ ~216 ns = 13.8 us.
  - Out (v @ attn^T): DoubleRow fp8: lhsT = vT jb-pair [128, 2, 128]
    (slab stride MUST be 128 - ISA dual-fp8 LDW restriction), rhs = aT
    [128, 2, 512].  One MM covers TWO j-blocks: 32 MMs/pair ~ 9 us.
    vT col 64 = ones => psum row 64 accumulates the softmax denominator.
  - exp: energies shifted by -3 (softmax-invariant) so exp in [e-12, 4.5].
    Split ACT (true exp -> fp8e5, bias=-3, ~1.37 us/[128,1024] tile) and
    DVE (Schraudolph in e5m2 bit domain: i8(round(A*e + B)), one
    tensor_scalar, ~1.2 us) - both read f32 PSUM, the hard wall (GPSIMD
    cannot access PSUM).
  - normalize on Pool (gpsimd), which cannot touch psum but is idle
    otherwise: yu = ACT copy psum->bf16; 1/s via bf16 magic seed + one
    Newton step; r broadcast over partitions via a DRAM round-trip DMA;
    y = x - yu*(-r) with the residual add in f32 (bf16 add costs 6e-3
    rel err; f32 keeps it at ~2e-3).  Last pair's chain runs on DVE.

Accuracy: attention term is ~1% of output; fp8 q/k/v + e5m2 attn weights
+ bf16 normalize cost ~2e-3 final relative error (tolerance 2e-2).
"""

import os
import sys

import numpy as np

sys.path.insert(0, "/opt/trn_rl_repo")

B, C, HH, WW = 8, 64, 64, 64
N = HH * WW  # 4096
D = 8  # qk channels
IC = 512  # i-chunk
N_IC = N // IC  # 8
JB = 128  # j-block
N_JB = N // JB  # 32
NP = N_JB // 2  # 16 jb-pairs

A5 = float(4.0 / np.log(2.0))
B5 = float(4 * 15.0 - 0.5)
SQK = float(0.25 * np.sqrt(A5))  # per-side scale: 16 reps * SQK^2 = A5
K16 = 0x7EF0  # bf16 reciprocal magic
PIPE = 3

# jb's whose exp runs on ACT (17, evenly spread); rest on DVE (15)
ACT_SET = frozenset(j for j in range(32) if (j * 17) // 32 != ((j + 1) * 17) // 32)

_CACHE = {}


def _build_program():
    import concourse.bass as bass
    import concourse.tile as tile
    from concourse import bacc, mybir
    from concourse.masks import make_identity

    f32 = mybir.dt.float32
    bf16 = mybir.dt.bfloat16
    i8 = mybir.dt.int8
    i16 = mybir.dt.int16
    fp8e4 = mybir.dt.float8e4
    fp8e5 = mybir.dt.float8e5
    EXP = mybir.ActivationFunctionType.Exp
    DR = mybir.MatmulPerfMode.DoubleRow
    MUL = mybir.AluOpType.mult
    ADD = mybir.AluOpType.add
    SUB = mybir.AluOpType.subtract

    nc = bacc.Bacc(
        "TRN2", target_bir_lowering=False, debug=False, enable_asserts=False
    )

    x_d = nc.dram_tensor("x", [C, N], f32, kind="ExternalInput").ap()
    wq_d = nc.dram_tensor("wq", [D, C], f32, kind="ExternalInput").ap()
    bq_d = nc.dram_tensor("bq", [D], f32, kind="ExternalInput").ap()
    wk_d = nc.dram_tensor("wk", [D, C], f32, kind="ExternalInput").ap()
    bk_d = nc.dram_tensor("bk", [D], f32, kind="ExternalInput").ap()
    wv_d = nc.dram_tensor("wv", [C, C], f32, kind="ExternalInput").ap()
    bv_d = nc.dram_tensor("bv", [C], f32, kind="ExternalInput").ap()
    gamma_d = nc.dram_tensor("gamma", [1], f32, kind="ExternalInput").ap()
    y_d = nc.dram_tensor("y", [C, N], f32, kind="ExternalOutput").ap()
    r_d = nc.dram_tensor("r_scr", [5, 2 * IC], bf16, kind="Internal").ap()

    with tile.TileContext(nc) as tc:
        from contextlib import ExitStack

        with ExitStack() as ctx:
            consts = ctx.enter_context(tc.tile_pool(name="consts", bufs=1))
            bigs = ctx.enter_context(tc.tile_pool(name="bigs", bufs=1))
            work = ctx.enter_context(tc.tile_pool(name="work", bufs=4))
            ypool = ctx.enter_context(tc.tile_pool(name="ypool", bufs=2))
            small = ctx.enter_context(tc.tile_pool(name="small", bufs=4))

            # ---------------- constants / weights prep ----------------
            ident = consts.tile([C, C], f32)
            make_identity(nc, ident)

            # warm the Exp activation table early (table load ~1.3us)
            warm = consts.tile([1, 8], f32)
            nc.scalar.activation(warm, ident[0:1, 0:8], EXP)

            mones = consts.tile([65, C], bf16)
            nc.vector.memset(mones, -1.0)

            gcol = consts.tile([65, 1], f32)
            nc.sync.dma_start(out=gcol, in_=gamma_d.to_broadcast([65, 1]))

            # x2: [65, N] bf16; rows 0:64 = x (casting DMA), row 64 = ones
            # (written via a DRAM round-trip DMA so no big memset gates the
            # projections; K=65 costs the HAM half-clock on prep matmuls
            # only).
            x2 = bigs.tile([65, N], bf16)
            for ic in range(N_IC):
                sl = slice(ic * IC, (ic + 1) * IC)
                nc.gpsimd.dma_start(out=x2[0:C, sl], in_=x_d[:, sl])
            pones = consts.tile([1, IC], bf16)
            nc.vector.memset(pones, 1.0)
            nc.sync.dma_start(out=r_d[4:5, 0:IC], in_=pones)
            nc.sync.dma_start(
                out=x2[C : C + 1, :],
                in_=r_d[4:5, 0:IC].to_broadcast([8, IC]),
            )
            # x_f32 for the final residual add
            xf32 = bigs.tile([C, N], f32)

            wq_sb = consts.tile([D, C], f32)
            wk_sb = consts.tile([D, C], f32)
            wv_sb = consts.tile([C, C], f32)
            nc.sync.dma_start(out=wq_sb, in_=wq_d)
            nc.sync.dma_start(out=wk_sb, in_=wk_d)
            nc.sync.dma_start(out=wv_sb, in_=wv_d)

            # fold gamma into wv; SQK into wq and wk (16 replicas of
            # (SQK q).(SQK k) summed over K give A5 * q.k, so the
            # Schraudolph multiply is free and ACT applies scale=1/A5)
            nc.vector.tensor_scalar_mul(wv_sb, wv_sb, gcol[0:C])
            nc.vector.tensor_scalar_mul(wq_sb, wq_sb, SQK)
            nc.vector.tensor_scalar_mul(wk_sb, wk_sb, SQK)

            wqT = consts.tile([65, 2 * C], bf16)
            wkT = consts.tile([65, 2 * C], bf16)
            wvT2 = consts.tile([65, C + 1], bf16)
            nc.gpsimd.memset(wvT2, 0.0)

            # vT: [128, NP, 2, 128] fp8e4; [j, p, s, 0:64] = gamma*v^T for
            # j-block 2p+s, col 64 = ones (denominator), cols 65:127 = 0
            # (psum rows 65:127 are never read, but keep them finite).
            vT = bigs.tile([JB, NP, 2, JB], fp8e4)
            nc.gpsimd.memset(vT, 0.0)
            nc.gpsimd.memset(vT[:, :, :, C : C + 1], 1.0)

            with tc.tile_pool(name="psum_x", bufs=4, space="PSUM") as psum_x:
                # biases staged on partition 64
                bst64 = consts.tile([65, 2 * D + C], f32)
                nc.sync.dma_start(out=bst64[C : C + 1, 0:D], in_=bq_d[None, :])
                nc.sync.dma_start(
                    out=bst64[C : C + 1, D : 2 * D], in_=bk_d[None, :]
                )
                nc.sync.dma_start(
                    out=bst64[C : C + 1, 2 * D :], in_=bv_d[None, :]
                )
                nc.vector.tensor_scalar_mul(
                    bst64[C : C + 1, 0 : 2 * D], bst64[C : C + 1, 0 : 2 * D],
                    SQK,
                )
                nc.vector.tensor_scalar_mul(
                    bst64[C : C + 1, 2 * D :], bst64[C : C + 1, 2 * D :],
                    gcol[C : C + 1],
                )

                nc.sync.dma_start(out=xf32[:, 0 : N // 2], in_=x_d[:, 0 : N // 2])
                nc.sync.dma_start(out=xf32[:, N // 2 :], in_=x_d[:, N // 2 :])

                wqT8 = consts.tile([65, D], bf16)
                pt = psum_x.tile([C, D], f32, tag="px")
                nc.tensor.transpose(pt, wq_sb, ident[0:D, 0:D])
                nc.vector.tensor_copy(out=wqT8[0:C, :], in_=pt)
                nc.vector.tensor_copy(
                    out=wqT8[C : C + 1, :], in_=bst64[C : C + 1, 0:D]
                )

                wkT8 = consts.tile([65, D], bf16)
                pt2 = psum_x.tile([C, D], f32, tag="px")
                nc.tensor.transpose(pt2, wk_sb, ident[0:D, 0:D])
                nc.vector.tensor_copy(out=wkT8[0:C, :], in_=pt2)
                nc.vector.tensor_copy(
                    out=wkT8[C : C + 1, :], in_=bst64[C : C + 1, D : 2 * D]
                )

                # replicate 16x across the 128 weight columns
                for w_dst, w_src in ((wqT, wqT8), (wkT, wkT8)):
                    sap = w_src[:]
                    rep = bass.AP(
                        tensor=sap.tensor,
                        offset=sap.offset,
                        ap=[sap.ap[0], [0, 16], sap.ap[1]],
                    )
                    nc.vector.tensor_copy(
                        out=w_dst[0:65].rearrange("p (g d) -> p g d", g=16),
                        in_=rep,
                    )

                # wvT2 [128, 65]: rows 0:64 = (gamma wv)^T, row 64 = gamma bv
                pt3 = psum_x.tile([C, C], f32, tag="px")
                nc.tensor.transpose(pt3, wv_sb, ident)
                nc.vector.tensor_copy(out=wvT2[0:C, 0:C], in_=pt3)
                nc.vector.tensor_copy(
                    out=wvT2[C : C + 1, 0:C], in_=bst64[C : C + 1, 2 * D :]
                )

                # ---------------- projections ----------------
                # q8/k8: [128, N] fp8e4 (16 partition-replicas of q/4, k/4)
                # double-width psum tiles amortize copy overhead; copies
                # alternate ACT/DVE so neither serializes the prep.
                q8 = bigs.tile([2 * C, N], fp8e4)
                k8 = bigs.tile([2 * C, N], fp8e4)
                for ic in range(N_IC):
                    sl = slice(ic * IC, (ic + 1) * IC)
                    pq = psum_x.tile([2 * C, IC], f32, tag="px")
                    nc.tensor.matmul(pq, wqT, x2[:, sl], start=True, stop=True)
                    nc.vector.tensor_copy(out=q8[:, sl], in_=pq)
                    pk = psum_x.tile([2 * C, IC], f32, tag="px")
                    nc.tensor.matmul(pk, wkT, x2[:, sl], start=True, stop=True)
                    nc.vector.tensor_copy(out=k8[:, sl], in_=pk)
                    pv = psum_x.tile([JB, 4 * 65], f32, tag="pv", bufs=4)
                    for j4 in range(4):
                        jc = ic * 4 + j4
                        nc.tensor.matmul(
                            pv[:, j4 * 65 : (j4 + 1) * 65],
                            x2[:, jc * JB : (jc + 1) * JB],
                            wvT2,
                            start=True,
                            stop=True,
                        )
                    nc.scalar.copy(
                        vT[:, 2 * ic : 2 * ic + 2, :, 0:C],
                        pv.rearrange("p (a b f) -> p a b f", a=2, b=2)[
                            :, :, :, 0:C
                        ],
                    )

            psum_e = ctx.enter_context(
                tc.tile_pool(name="psum_e", bufs=3, space="PSUM")
            )
            psum_o = ctx.enter_context(
                tc.tile_pool(name="psum_o", bufs=1, space="PSUM")
            )

            # ---------------- main attention loop ----------------
            # Deferred normalize: pair pr's chain is emitted early in pair
            # pr+1 (Pool + DMA only; nothing the PE waits on).  The last
            # pair's chain runs on DVE after the loop.
            norm_q = []

            def emit_norm(yu, sl2, pr):
                # mid-loop normalize (pairs 0..2): seed on DVE (i16 TS is
                # DVE-only), Newton + big TTs on Pool, partition-broadcast
                # of r via a DRAM round-trip.
                # r0 = +1/s seed: bitcast_bf16(K16 - int16(s_bits))
                r0i = small.tile([C + 1, 2 * IC], i16, tag="r0")
                nc.vector.tensor_scalar(
                    r0i[C : C + 1, :],
                    yu[C : C + 1, :].bitcast(i16),
                    -1.0,
                    float(K16),
                    op0=MUL,
                    op1=ADD,
                )
                r0 = r0i.bitcast(bf16)
                # one Newton step, lands NEGATED: rn = (s*r0 - 2)*r0 = -1/s
                t1 = small.tile([C + 1, 2 * IC], bf16, tag="t1")
                nc.gpsimd.tensor_tensor(
                    out=t1[C : C + 1, :], in0=yu[C : C + 1, :],
                    in1=r0[C : C + 1, :], op=MUL,
                )
                u = small.tile([C + 1, 2 * IC], bf16, tag="u")
                nc.gpsimd.tensor_scalar(
                    u[C : C + 1, :], t1[C : C + 1, :], 1.0, -2.0,
                    op0=MUL, op1=ADD,
                )
                rn = small.tile([C + 1, 2 * IC], bf16, tag="rn")
                nc.gpsimd.tensor_tensor(
                    out=rn[C : C + 1, :], in0=u[C : C + 1, :],
                    in1=r0[C : C + 1, :], op=MUL,
                )
                # broadcast -r over 64 partitions via DRAM round-trip
                nc.sync.dma_start(out=r_d[pr : pr + 1, :], in_=rn[C : C + 1, :])
                rb = small.tile([C, 2 * IC], bf16, tag="rb")
                nc.sync.dma_start(
                    out=rb, in_=r_d[pr : pr + 1, :].to_broadcast([C, 2 * IC])
                )
                # t = yu * (-r);  y = x - t  (f32 residual add)
                t2 = small.tile([C, 2 * IC], bf16, tag="t2")
                nc.gpsimd.tensor_tensor(out=t2, in0=yu[0:C, :], in1=rb, op=MUL)
                y_sb = ypool.tile([C, 2 * IC], f32)
                nc.gpsimd.tensor_tensor(
                    out=y_sb, in0=xf32[:, sl2], in1=t2, op=SUB
                )
                nc.sync.dma_start(out=y_d[:, sl2], in_=y_sb)

            def emit_norm_tail(yu, sl2):
                # last pair: latency-optimal.  Two independent half-chains
                # (the idle-PE downclock halves tail op rates, so overlap
                # ACT/DVE/PE work).  Magic seed only, K=1 matmul broadcast.
                for h in range(2):
                    hs = slice(h * IC, (h + 1) * IC)
                    hs2 = slice(sl2.start + h * IC, sl2.start + (h + 1) * IC)
                    r0i = small.tile([C + 1, IC], i16, tag=f"r0t{h}")
                    nc.vector.tensor_scalar(
                        r0i[C : C + 1, :],
                        yu[C : C + 1, hs].bitcast(i16),
                        -1.0,
                        float(K16),
                        op0=MUL,
                        op1=ADD,
                    )
                    r0 = r0i.bitcast(bf16)
                    rb_ps = psum_e.tile([JB, 2 * IC], f32, tag="e_ps")
                    nc.tensor.matmul(
                        rb_ps[0:C, 0:IC], mones[C : C + 1, :],
                        r0[C : C + 1, :], start=True, stop=True,
                    )
                    t2 = small.tile([C, IC], bf16, tag=f"t2t{h}")
                    nc.vector.tensor_tensor(
                        out=t2, in0=yu[0:C, hs], in1=rb_ps[0:C, 0:IC], op=MUL
                    )
                    y_sb = ypool.tile([C, IC], f32)
                    nc.vector.tensor_tensor(
                        out=y_sb, in0=xf32[:, hs2], in1=t2, op=SUB
                    )
                    nc.sync.dma_start(out=y_d[:, hs2], in_=y_sb)

            # Flat jb-stream across all 4 i-chunk pairs: the next pair's
            # energy matmuls fill the PE wait on the previous pair's tail
            # exps (no per-pair boundary stall).
            NPAIRS = N_IC // 2
            NT = NPAIRS * N_JB
            o_tiles = {}
            a_tiles = {}
            for g in range(NT + PIPE + 1):
                pr, jb = divmod(g, N_JB)
                if norm_q and jb == 1:
                    norm_q.pop(0)()
                if g < NT:
                    slA = slice((2 * pr) * IC, (2 * pr + 1) * IC)
                    slB = slice((2 * pr + 1) * IC, (2 * pr + 2) * IC)
                    e_ps = psum_e.tile([JB, 2 * IC], f32)
                    kblk = k8[:, jb * JB : (jb + 1) * JB]
                    nc.tensor.matmul(
                        e_ps[:, 0:IC], kblk, q8[:, slA],
                        start=True, stop=True,
                    )
                    nc.tensor.matmul(
                        e_ps[:, IC : 2 * IC], kblk, q8[:, slB],
                        start=True, stop=True,
                    )
                    p = jb // 2
                    if jb % 2 == 0:
                        aT_new = work.tile([JB, 2, 2 * IC], fp8e5, tag="aT")
                        a_tiles[(pr, p)] = aT_new
                    if jb in ACT_SET:
                        nc.scalar.activation(
                            a_tiles[(pr, p)][:, jb % 2, :], e_ps, EXP,
                            scale=float(1.0 / A5),
                        )
                    else:
                        nc.vector.tensor_scalar(
                            a_tiles[(pr, p)][:, jb % 2, :].bitcast(i8),
                            e_ps,
                            B5,
                            None,
                            op0=ADD,
                        )
                go = g - PIPE
                if 0 <= go < NT:
                    pro, jo = divmod(go, N_JB)
                    if jo % 2 == 1:
                        p = jo // 2
                        if p == 0:
                            o_new = psum_o.tile(
                                [2 * C, 2 * IC], f32, tag="op"
                            )
                            o_tiles[pro] = o_new
                        o_ps = o_tiles[p if False else pro]
                        aT = a_tiles.pop((pro, p))
                        nc.tensor.matmul(
                            o_ps[:, 0:IC],
                            vT[:, p],
                            aT[:, :, 0:IC],
                            start=(p == 0),
                            stop=(p == NP - 1),
                            perf_mode=DR,
                        )
                        nc.tensor.matmul(
                            o_ps[:, IC : 2 * IC],
                            vT[:, p],
                            aT[:, :, IC : 2 * IC],
                            start=(p == 0),
                            stop=(p == NP - 1),
                            perf_mode=DR,
                        )
                        if p == NP - 1:
                            # evacuate rows 0:65 to bf16 on ACT; frees the
                            # psum banks for the next pair's accumulator
                            o_done = o_tiles.pop(pro)
                            yu = small.tile([C + 1, 2 * IC], bf16, tag="yu")
                            nc.scalar.copy(yu, o_done[0 : C + 1, :])
                            sl2 = slice(
                                (2 * pro) * IC, (2 * pro + 2) * IC
                            )
                            if pro < NPAIRS - 1:
                                norm_q.append(
                                    lambda yu=yu, sl2=sl2, pro=pro: emit_norm(
                                        yu, sl2, pro
                                    )
                                )
                            else:
                                emit_norm_tail(yu, sl2)

            while norm_q:
                norm_q.pop(0)()

    nc.compile()
    return nc


def _get_program():
    if "nc" not in _CACHE:
        _CACHE["nc"] = _build_program()
    return _CACHE["nc"]


def kernel(**inputs) -> np.ndarray:
    import time

    nc = _get_program()
    from concourse.bass_utils import run_bass_kernel_spmd

    x = np.ascontiguousarray(np.asarray(inputs["x"], dtype=np.float32))
    shared = {
        k: np.ascontiguousarray(np.asarray(inputs[k], dtype=np.float32))
        for k in ("wq", "bq", "wk", "bk", "wv", "bv", "gamma")
    }
    in_maps = [
        {"x": x[b].reshape(C, N).copy(), **shared} for b in range(B)
    ]
    # the axon-tunneled device occasionally reports a transient
    # NRT_EXEC_UNIT_UNRECOVERABLE; a retry on a fresh execution succeeds
    last_err = None
    for attempt in range(4):
        try:
            res = run_bass_kernel_spmd(nc, in_maps, list(range(B)))
            break
        except Exception as e:  # noqa: BLE001
            last_err = e
            time.sleep(2.0 * (attempt + 1))
    else:
        raise last_err
    out = np.stack(
        [res.results[b]["y"].reshape(C, HH, WW) for b in range(B)], axis=0
    )
    return out.astype(np.float32)


if __name__ == "__main__":
    rng = np.random.default_rng(0)
    inputs = {
        "x": rng.standard_normal((B, C, HH, WW), dtype=np.float32),
        "wq": rng.standard_normal((D, C), dtype=np.float32) * 0.05,
        "bq": rng.standard_normal((D,), dtype=np.float32) * 0.05,
        "wk": rng.standard_normal((D, C), dtype=np.float32) * 0.05,
        "bk": rng.standard_normal((D,), dtype=np.float32) * 0.05,
        "wv": rng.standard_normal((C, C), dtype=np.float32) * 0.05,
        "bv": rng.standard_normal((C,), dtype=np.float32) * 0.05,
        "gamma": rng.standard_normal((1,), dtype=np.float32),
    }
    out = kernel(**inputs)
    print("out", out.shape, out.dtype, float(np.abs(out).max()))
